# revision 21
# baseline (speedup 1.0000x reference)
"""Trainium2 Bass kernel for nn_GumbelLayer: out = sigmoid((x@W.T + b + g1 - g2)/T).

g_i = -log(-log(u_i)), T = 0.1. Shapes: x,u1,u2,out [16384,1024]; W [1024,1024]; b [1024].
Data-parallel over 8 NeuronCores: each core handles 2048 batch rows; W/b replicated.

Device-side math per core (2048 rows = 16 tiles of 128 partitions):
  a_i  = Ln(u_i)            (ACT, fp32)
  d_i  = Ln(-a_i)           (ACT, fp16 out)
  s    = d2 - d1 + b        (DVE fp16 2x sub + per-tile adds)
  psum = x @ W.T + s        (PE fp16; s added via identity-matmul accumulate)
  t    = Exp(-10 * psum)    (ACT reads PSUM; same table set as Ln -> 1 table load)
  q    = min(t, 1e6) + 1    (DVE fused tensor_scalar; kills Exp overflow edge)
  r    = 1/q                (DVE reciprocal_approx_fast, ~51 ULP)
  out  = fp16(r)            (cast on the wire via SWDGE dma)

out = 1/(1+e^-10z) = sigmoid(10z) exactly; clamp only distorts already-saturated
outputs (|err| <= 1e-6). Engine budget/core: ACT ~76us, PE ~65us, DVE ~48us,
DMA 26MiB ~76us -> ACT/DMA co-critical.

Orchestration:
- One ACT table set for everything: Ln chunks run one chunk ahead of the
  matmul/Exp stream so ACT never stalls on PSUM.
- u1/u2/xt ride the sync HWDGE ring; wts/consts + cast-out-stores ride the
  gpsimd SWDGE ring (cast fp32->fp16 during store halves out traffic).
- x/W fp16 on the wire and in the matmul.
"""
import sys

if '/opt/trn_rl_repo' not in sys.path:
    sys.path.insert(0, '/opt/trn_rl_repo')

import numpy as np

import concourse.bass as bass
import concourse.tile as tile
from concourse import bacc, mybir
from concourse.bass_utils import run_bass_kernel_spmd
from concourse.hw_specs import get_activation_tables
from concourse.tile_rust import add_dep_helper

B, D = 16384, 1024
NCORES = 8
BS = B // NCORES          # 2048 rows per core
P = 128
BT = BS // P              # 16 row-tiles per core
KT = D // P               # 8 contraction chunks
N_HALF = 512              # matmul moving free-dim (one PSUM bank)
LN_CHUNKS = (1, 1, 2, 2, 2, 2, 2, 2, 1, 1)   # row-tiles per Ln chunk
MM_CH = 2                 # row-tiles per matmul/Exp chunk
TEMP_INV = 10.0           # 1/T
CLAMP = 1.0e6             # cap on e^{-10z} before 1/(1+t)

f32 = mybir.dt.float32
f16 = mybir.dt.float16
AF = mybir.ActivationFunctionType
ALU = mybir.AluOpType


def build_kernel():
    nc = bacc.Bacc("TRN2", target_bir_lowering=False, debug=False,
                   num_devices=NCORES)
    # xt[t, p, j*128+c] = x[t*128+c, j*128+p]  (pre-transposed on host, fp16)
    xt = nc.dram_tensor("xt", [BT, P, D], f16, kind="ExternalInput")
    u1 = nc.dram_tensor("u1", [BS, D], f32, kind="ExternalInput")
    u2 = nc.dram_tensor("u2", [BS, D], f32, kind="ExternalInput")
    wt = nc.dram_tensor("wt", [D, D], f16, kind="ExternalInput")   # W.T
    bb = nc.dram_tensor("bb", [P, D], f16, kind="ExternalInput")   # b row-bcast
    ident = nc.dram_tensor("ident", [P, P], f16, kind="ExternalInput")
    out = nc.dram_tensor("out", [BS, D], f16, kind="ExternalOutput")

    with tile.TileContext(nc) as tc:
        _body(tc, nc, xt, u1, u2, wt, bb, ident, out)
    nc.compile()
    return nc


def _act(nc, tload, *args, **kwargs):
    # every ACTIVATE is ordered after the explicit combined-table load so the
    # fixpoint pass never inserts per-function (Ln<->Exp thrash) reloads
    ins = nc.scalar.activation(*args, **kwargs)
    add_dep_helper(ins.ins, tload.ins, sync=False, reason="act table preload")
    return ins


def _body(tc, nc, xt, u1, u2, wt, bb, ident, out):
    ch_max = max(LN_CHUNKS)
    # explicit load of the one table set holding BOTH Ln and Exp
    tabs = get_activation_tables(nc.m.arch)
    set_id = list(tabs.keys()).index("natural_log_exp_and_others")
    tload = nc.scalar.add_instruction(
        mybir.InstLoadActFuncSet(name="act_tbl_preload",
                                 act_func_set_id=set_id))
    with (
        tc.tile_pool(name="const", bufs=1) as cpool,
        tc.tile_pool(name="wts", bufs=1) as wpool,
        tc.tile_pool(name="uin", bufs=3) as upool,
        tc.tile_pool(name="lna", bufs=2) as apool,
        tc.tile_pool(name="lnd", bufs=2) as dpool,
        tc.tile_pool(name="gum", bufs=2) as spool,
        tc.tile_pool(name="xin", bufs=3) as xpool,
        tc.tile_pool(name="expt", bufs=2) as tpool,
        tc.tile_pool(name="rout", bufs=3) as rpool,
        tc.tile_pool(name="ps", bufs=2, space="PSUM") as pspool,
    ):
        bbt = cpool.tile([P, D], f16)
        nc.gpsimd.dma_start(bbt[:], bb.ap()[:])
        idt = cpool.tile([P, P], f16)
        nc.gpsimd.dma_start(idt[:], ident.ap()[:])

        # W.T resident in SBUF: wts[p, j, o] = W.T[j*128+p, o], fp16.
        # First half upfront on the gpsimd ring; second half rides the sync
        # ring behind chunk 0's u so the startup burst doesn't starve u.
        wts = wpool.tile([P, KT, D], f16)
        wtr = wt.ap().rearrange("(j p) o -> p j o", p=P)
        for j in range(2):
            nc.gpsimd.dma_start(wts[:, j, :], wtr[:, j, :])

        u1r = u1.ap().rearrange("(n p) d -> p n d", p=P)   # [128, 16, 1024]
        u2r = u2.ap().rearrange("(n p) d -> p n d", p=P)
        xtr = xt.ap().rearrange("t p d -> p t d")
        outr = out.ap().rearrange("(n p) d -> p n d", p=P)

        def emit_dma_chunk(t0, ch):
            # u1 and u2 side by side in one tile -> single Ln covers both
            sl = slice(t0, t0 + ch)
            uc = upool.tile([P, 2, ch_max, D], f32, tag="u")
            nc.sync.dma_start(uc[:, 0, :ch, :], u1r[:, sl, :])
            nc.sync.dma_start(uc[:, 1, :ch, :], u2r[:, sl, :])
            xc = xpool.tile([P, ch_max, D], f16, tag="x")
            nc.sync.dma_start(xc[:, :ch, :], xtr[:, sl, :])
            return uc, xc

        def emit_inner(uc, ch, split=False):
            a12 = apool.tile([P, 2, ch_max, D], f32, tag="a")
            if split:
                # first chunk: separate u1/u2 instrs so the first starts as
                # soon as u1 alone has landed
                _act(nc, tload, a12[:, 0, :ch, :], uc[:, 0, :ch, :], AF.Ln)
                _act(nc, tload, a12[:, 1, :ch, :], uc[:, 1, :ch, :], AF.Ln)
            else:
                _act(nc, tload, a12[:, :, :ch, :], uc[:, :, :ch, :], AF.Ln)
            return a12

        def emit_outer(a12, ch):
            d12 = dpool.tile([P, 2, ch_max, D], f16, tag="d")
            _act(nc, tload, d12[:, :, :ch, :], a12[:, :, :ch, :], AF.Ln,
                 scale=-1.0)
            # s = d2 - d1 + b   (fp16, 2x DVE)
            sc = spool.tile([P, ch_max, D], f16, tag="s")
            nc.vector.tensor_sub(sc[:, :ch, :], d12[:, 1, :ch, :],
                                 d12[:, 0, :ch, :])
            for k in range(ch):
                nc.vector.tensor_add(sc[:, k, :], sc[:, k, :], bbt[:])
            return sc

        def emit_mm_chunk(t0, mch, sc, xc, koff):
            # mch row-tiles: matmul into one [P, mch*1024] psum tile,
            # then Exp -> clamp+1 -> reciprocal -> store (cast fp16)
            psum = pspool.tile([P, MM_CH, D], f32)
            for k in range(mch):
                xs = xc[:, koff + k, :]
                for j in range(KT):
                    for n in range(2):
                        nsl = slice(n * N_HALF, (n + 1) * N_HALF)
                        nc.tensor.matmul(
                            psum[:, k, nsl],
                            xs[:, j * P:(j + 1) * P],
                            wts[:, j, nsl],
                            start=(j == 0), stop=False)
                for n in range(2):
                    nsl = slice(n * N_HALF, (n + 1) * N_HALF)
                    nc.tensor.matmul(
                        psum[:, k, nsl],
                        idt[:],
                        sc[:, koff + k, nsl],
                        start=False, stop=True)
            tt = tpool.tile([P, MM_CH, D], f32, tag="t")
            _act(nc, tload, tt[:, :mch, :], psum[:, :mch, :], AF.Exp,
                 scale=-TEMP_INV)
            nc.vector.tensor_scalar(tt[:, :mch, :], tt[:, :mch, :],
                                    CLAMP, 1.0, ALU.min, ALU.add)
            rt = rpool.tile([P, MM_CH, D], f32, tag="r")
            nc.vector.reciprocal_approx_fast(rt[:, :mch, :], tt[:, :mch, :])
            nc.gpsimd.dma_start(outr[:, t0:t0 + mch, :], rt[:, :mch, :])

        chunk_starts = []
        t0 = 0
        for ch in LN_CHUNKS:
            chunk_starts.append((t0, ch))
            t0 += ch
        n_ch = len(LN_CHUNKS)
        # 3-stage software pipeline with one chunk of DMA run-ahead:
        #   dma(ci) | innerLn(ci-1) | outerLn+s+matmul+exp+recip+store(ci-2)
        # Adjacent ACT instructions stay independent, hiding SBUF write-acks.
        st = {}
        for ci in range(n_ch + 2):
            if ci < n_ch:
                lt0, lch = chunk_starts[ci]
                uc, xc = emit_dma_chunk(lt0, lch)
                st[ci] = [uc, xc, None]
            if ci == 0:
                # bulk of the weights behind chunk 0's u on the sync ring
                for j in range(2, KT):
                    nc.sync.dma_start(wts[:, j, :], wtr[:, j, :])
            if 1 <= ci <= n_ch:
                lch = chunk_starts[ci - 1][1]
                st[ci - 1][2] = emit_inner(st[ci - 1][0], lch, split=(ci == 1))
            if ci >= 2:
                mt0, mch_tot = chunk_starts[ci - 2]
                uc, xc, a12 = st.pop(ci - 2)
                sc = emit_outer(a12, mch_tot)
                for o in range(0, mch_tot, MM_CH):
                    m = min(MM_CH, mch_tot - o)
                    emit_mm_chunk(mt0 + o, m, sc, xc, o)


_NC_CACHE = None


def _get_nc():
    global _NC_CACHE
    if _NC_CACHE is None:
        _NC_CACHE = build_kernel()
    return _NC_CACHE


def _prep_core_inputs(x_c, u1_c, u2_c, wt_np, bb_np, id_np):
    # xt[t, p, j*128+c] = x[t*128+c, j*128+p]
    xt_c = np.ascontiguousarray(
        x_c.reshape(BT, P, KT, P).transpose(0, 3, 2, 1).reshape(BT, P, D)
        .astype(np.float16))
    return {"xt": xt_c, "u1": np.ascontiguousarray(u1_c),
            "u2": np.ascontiguousarray(u2_c), "wt": wt_np, "bb": bb_np,
            "ident": id_np}


def run(x, u1, u2, W, b, trace=False, **trace_kwargs):
    nc = _get_nc()
    x = np.asarray(x, dtype=np.float32)
    u1 = np.asarray(u1, dtype=np.float32)
    u2 = np.asarray(u2, dtype=np.float32)
    wt_np = np.ascontiguousarray(
        np.asarray(W, dtype=np.float32).T.astype(np.float16))
    bb_np = np.ascontiguousarray(np.broadcast_to(
        np.asarray(b, dtype=np.float32).astype(np.float16).reshape(1, D),
        (P, D)))
    id_np = np.eye(P, dtype=np.float16)
    in_maps = []
    for c in range(NCORES):
        sl = slice(c * BS, (c + 1) * BS)
        in_maps.append(
            _prep_core_inputs(x[sl], u1[sl], u2[sl], wt_np, bb_np, id_np))
    res = run_bass_kernel_spmd(nc, in_maps, list(range(NCORES)),
                               trace=trace, **trace_kwargs)
    out = np.concatenate([res.results[c]["out"] for c in range(NCORES)], axis=0)
    return out.astype(np.float32), res


def kernel(x, u1, u2, W, b, with_grad=None):
    out, _ = run(x, u1, u2, W, b)
    return out
